# revision 7
# baseline (speedup 1.0000x reference)
"""Trainium2 Bass kernel for MHSA with relative position bias (nn_MHSARPB).

Problem (hardcoded): x (8, 32, 32, 512), qkv_w (1536, 512), qkv_b (1536,),
rpb (16, 63, 63), proj_w (512, 512), proj_b (512,). Output (8, 32, 32, 512) f32.

Strategy: tensor-parallel over the 16 heads -> 2 heads per core on 8 cores.
Each core computes q/k/v for its 2 heads over all 8*1024 tokens, the full
attention for its (8 batches x 2 heads) pairs, and a partial projection
output (contraction over its 64 channels). The host sums the 8 partial
projection outputs and adds proj_b.

Device dataflow is entirely in "transposed token space":
  - qkv:   qkT/vT = W_h @ x^T       (x^T prepared on host, fp16)
  - S^T:   (m keys on partitions, n queries free) = k^T-slices.T @ q^T
           via 4-way row-packed K=32 matmuls (tile_position=(32u, 0))
  - bias:  bias^T[m, n] = rpb[h, 31 + i2(m) - i1(n), 31 + j2(m) - j1(n)]
           (exact identity). Applied as E = exp(S^T) * exp(bias^T) with the
           exp(bias^T) table precomputed on host (resident fp16, DVE 2x mul).
  - softmax: exp on ScalarE reading (128, 2048) across 4 PSUM banks; no
           max-subtraction (|logits| <= ~2, fp16-safe); denominators come
           for free from a ones-column appended to v in the AV matmul.
  - AV:    out^T (d, n) accumulated over 8 key-chunks, 2 heads col-packed
           (tile_position=(0, 64*hi)), v in natural token-major layout
           produced by on-chip DMA xbar transposes.
  - proj:  partial out^T (co, t) = projT_zeropad.T @ Y_norm, K=128 with
           zero weight rows annihilating the denominator/junk rows.

Phases are interleaved per batch so qkv matmuls/DMAs overlap the previous
batch's softmax; DMA work is spread over SWDGE (gpsimd) and both HWDGE rings.
"""
import sys

sys.path.insert(0, "/opt/trn_rl_repo")

import contextlib
import numpy as np
import concourse.bass as bass
import concourse.bacc as bacc
import concourse.tile as tile
from concourse import mybir
from concourse.bass_utils import run_bass_kernel_spmd

FP16 = mybir.dt.float16
FP32 = mybir.dt.float32
EXP = mybir.ActivationFunctionType.Exp

B, S, C, NH = 8, 32, 512, 16
N = S * S            # 1024 tokens per image
T = B * N            # 8192 tokens
D = C // NH          # 32 head dim
SCALE = D ** -0.5
N_CORES = 8

_CACHE = {}


def build_nc(repeat=1):
    nc = bacc.Bacc("TRN2", target_bir_lowering=False, debug=False)

    xT = nc.dram_tensor("xT", [C, T], FP16, kind="ExternalInput")
    wqkT = nc.dram_tensor("wqkT", [4, 128, 128], FP16, kind="ExternalInput")
    wvT = nc.dram_tensor("wvT", [4, 128, 64], FP16, kind="ExternalInput")
    bqk = nc.dram_tensor("bqk", [128, 1], FP32, kind="ExternalInput")
    bv2 = nc.dram_tensor("bv2", [128, 1], FP32, kind="ExternalInput")
    expb = nc.dram_tensor("expb", [128, 16384], FP16, kind="ExternalInput")
    projT = nc.dram_tensor("projT", [128, 512], FP16, kind="ExternalInput")
    outT = nc.dram_tensor("outT", [C, T], FP16, kind="ExternalOutput")
    den_scr = nc.dram_tensor("den_scr", [2, 8192], FP16)
    denr_scr = nc.dram_tensor("denr_scr", [2, 8192], FP16)

    with tile.TileContext(nc) as tc:
        with (
            tc.For_i(0, repeat, 1) if repeat > 1 else contextlib.nullcontext(),
            tc.tile_pool(name="consts", bufs=1) as consts,
            tc.tile_pool(name="big", bufs=1) as big,
            tc.tile_pool(name="xin", bufs=3) as xin,
            tc.tile_pool(name="qpool", bufs=4) as qpool,
            tc.tile_pool(name="epool", bufs=8) as epool,
            tc.tile_pool(name="stpool", bufs=4) as stpool,
            tc.tile_pool(name="opool", bufs=4) as opool,
        ):
            # ---- constants -------------------------------------------------
            wqk_sb = consts.tile([128, 4 * 128], FP16, tag="wqk_sb")
            nc.sync.dma_start(
                out=wqk_sb.rearrange("p (kc f) -> p kc f", kc=4),
                in_=wqkT[:].transpose([1, 0, 2]),
            )
            wv_sb = consts.tile([128, 4 * 64], FP16, tag="wv_sb")
            nc.sync.dma_start(
                out=wv_sb.rearrange("p (kc f) -> p kc f", kc=4),
                in_=wvT[:].transpose([1, 0, 2]),
            )
            bqk_sb = consts.tile([128, 1], FP32, tag="bqk_sb")
            nc.sync.dma_start(out=bqk_sb[:], in_=bqk[:])
            bv2_sb = consts.tile([128, 1], FP32, tag="bv2_sb")
            nc.sync.dma_start(out=bv2_sb[:], in_=bv2[:])
            expb_sb = consts.tile([128, 16384], FP16, tag="expb_sb")
            nc.sync.dma_start(out=expb_sb[:], in_=expb[:])
            projT_sb = consts.tile([128, 512], FP16, tag="projT_sb")
            nc.sync.dma_start(out=projT_sb[:], in_=projT[:])

            # ---- persistent big tensors -----------------------------------
            qkT_sb = big.tile([128, T], FP16, tag="qkT_sb")       # q rows 0-63, k rows 64-127
            vT_sb = big.tile([128, T // 2], FP16, tag="vT_sb")    # 2-chunk col-packed v
            kT_pack = big.tile([128, 16 * 256], FP16, tag="kT_pack")
            v_nat = big.tile([128, T], FP16, tag="v_nat")         # (pair, j) 64-col blocks
            y_sb = big.tile([128, T], FP16, tag="y_sb")
            r_bcast = big.tile([128, T], FP16, tag="r_bcast")

            nc.gpsimd.memset(v_nat[:], 0.0)
            nc.gpsimd.memset(
                v_nat.rearrange("p (blk cc) -> p blk cc", cc=64)[:, :, 32:33], 1.0
            )
            v5 = v_nat.rearrange("p (hi b j col) -> p hi b j col", hi=2, b=8, j=8)

            with (
                tc.tile_pool(name="ps_s", bufs=1, space="PSUM") as ps_s,
                tc.tile_pool(name="ps_qk", bufs=1, space="PSUM") as ps_qk,
                tc.tile_pool(name="ps_v", bufs=1, space="PSUM") as ps_v,
                tc.tile_pool(name="ps_av", bufs=2, space="PSUM") as ps_av,
            ):
                for b in range(8):
                    # ---- qkv for this batch's two 512-token chunks --------
                    psv = ps_v.tile([128, 512], FP32, tag="psv")
                    for cc in range(2):
                        c = 2 * b + cc
                        xt = xin.tile([128, 2048], FP16, tag="xt")
                        nc.sync.dma_start(
                            out=xt.rearrange("p (kc f) -> p kc f", kc=4),
                            in_=xT.rearrange("(kc p) t -> p kc t", p=128)[
                                :, :, c * 512 : (c + 1) * 512
                            ],
                        )
                        psqk = ps_qk.tile([128, 512], FP32, tag="psqk")
                        for kc in range(4):
                            nc.tensor.matmul(
                                psqk[:],
                                wqk_sb[:, kc * 128 : (kc + 1) * 128],
                                xt[:, kc * 512 : (kc + 1) * 512],
                                start=(kc == 0), stop=(kc == 3),
                            )
                        nc.vector.tensor_scalar_add(
                            qkT_sb[:, c * 512 : (c + 1) * 512], psqk[:], bqk_sb[:]
                        )
                        for kc in range(4):
                            nc.tensor.matmul(
                                psv[64 * cc : 64 * cc + 64, :],
                                wv_sb[:, kc * 64 : (kc + 1) * 64],
                                xt[:, kc * 512 : (kc + 1) * 512],
                                start=(kc == 0), stop=(kc == 3),
                                tile_position=(0, 64 * cc),
                            )
                    nc.vector.tensor_scalar_add(
                        vT_sb[:, b * 512 : (b + 1) * 512], psv[:], bv2_sb[:]
                    )

                    # ---- rearrange: kT_pack, v_nat, qrep (SWDGE + ACT ring)
                    for hi in range(2):
                        p_idx = hi * 8 + b
                        for u in range(4):
                            nc.sync.dma_start(
                                out=kT_pack[32 * u : 32 * u + 32,
                                            p_idx * 256 : p_idx * 256 + 256],
                                in_=qkT_sb[64 + 32 * hi : 64 + 32 * hi + 32,
                                           b * 1024 + u * 256 : b * 1024 + u * 256 + 256],
                            )
                    for wl in range(4):
                        w = 4 * b + wl
                        j0 = wl
                        stg = stpool.tile([128, 128], FP16, tag="stg")
                        nc.sync.dma_start_transpose(
                            out=stg[:], in_=vT_sb[:, w * 128 : (w + 1) * 128]
                        )
                        stg4 = stg.rearrange("p (par hi d) -> p par hi d", par=2, hi=2)
                        for hi in range(2):
                            nc.gpsimd.dma_start(
                                out=v5[:, hi, b, j0::4, 0:32],
                                in_=stg4[:, :, hi, :],
                            )
                    qreps = []
                    for hi in range(2):
                        q_t = qpool.tile([128, 1024], FP16, tag="qrep")
                        for u in range(4):
                            nc.sync.dma_start(
                                out=q_t[32 * u : 32 * u + 32, :],
                                in_=qkT_sb[32 * hi : 32 * hi + 32,
                                           b * 1024 : (b + 1) * 1024],
                            )
                        qreps.append(q_t)

                    # ---- attention for this batch -------------------------
                    for half in range(2):
                        e_ts = []
                        for hi in range(2):
                            p_idx = hi * 8 + b
                            e_jjs = []
                            for jj in range(2):
                                sps = ps_s.tile([128, 2048], FP32, tag="sps")
                                spsv = sps.rearrange("p (u n) -> p u n", n=512)
                                for u in range(4):
                                    nc.tensor.matmul(
                                        spsv[:, u, :],
                                        kT_pack[32 * u : 32 * u + 32,
                                                p_idx * 256 + jj * 128 :
                                                p_idx * 256 + jj * 128 + 128],
                                        qreps[hi][32 * u : 32 * u + 32,
                                                  half * 512 : (half + 1) * 512],
                                        start=True, stop=True,
                                        tile_position=(32 * u, 0),
                                    )
                                # separate E tile per quarter: exp(jj=1) does not
                                # wait for the expb-multiply of jj=0
                                e_q = epool.tile([128, 2048], FP16, tag="E")
                                eqv = e_q.rearrange("p (u n) -> p u n", n=512)
                                # two 2-bank exps -> QK of the next quarter can
                                # overwrite banks 0-1 while banks 2-3 still read
                                nc.scalar.activation(eqv[:, 0:2, :], spsv[:, 0:2, :], EXP)
                                nc.scalar.activation(eqv[:, 2:4, :], spsv[:, 2:4, :], EXP)
                                blk = (hi * 2 + half) * 2 + jj
                                nc.vector.tensor_mul(
                                    e_q[:], e_q[:],
                                    expb_sb[:, blk * 2048 : (blk + 1) * 2048],
                                )
                                e_jjs.append(e_q)
                            e_ts.append(e_jjs)
                        av = ps_av.tile([128, 512], FP32, tag="av")
                        for j in range(8):
                            for hi in range(2):
                                p_idx = hi * 8 + b
                                nc.tensor.matmul(
                                    av[64 * hi : 64 * hi + 64, :],
                                    v_nat[:, (p_idx * 8 + j) * 64 :
                                          (p_idx * 8 + j) * 64 + 64],
                                    e_ts[hi][j % 2][:, (j // 2) * 512 :
                                                    (j // 2) * 512 + 512],
                                    start=(j == 0), stop=(j == 7),
                                    tile_position=(0, 64 * hi),
                                )
                        nc.vector.tensor_copy(
                            y_sb[:, b * 1024 + half * 512 :
                                 b * 1024 + half * 512 + 512],
                            av[:],
                        )

            # ---- normalize + projection (attention PSUM pools closed) -----
            for hi in range(2):
                nc.sync.dma_start(
                    out=den_scr[hi : hi + 1, :],
                    in_=y_sb[32 + 64 * hi : 33 + 64 * hi, :],
                )
            den_g = stpool.tile([128, 128], FP32, tag="den_g")
            nc.gpsimd.dma_start(
                out=den_g[:], in_=den_scr[:].rearrange("h (z n) -> (h z) n", n=128)
            )
            den_r = stpool.tile([128, 128], FP32, tag="den_r")
            nc.vector.reciprocal(den_r[:], den_g[:])
            nc.gpsimd.dma_start(
                out=denr_scr[:].rearrange("h (z n) -> (h z) n", n=128), in_=den_r[:]
            )
            for hi in range(2):
                nc.sync.dma_start(
                    out=r_bcast[64 * hi : 64 * hi + 64, :],
                    in_=bass.AP(
                        tensor=denr_scr,
                        offset=hi * 8192,
                        ap=[[0, 64], [1, 8192]],
                    ),
                )
            nc.vector.tensor_mul(y_sb[:], y_sb[:], r_bcast[:])

            with tc.tile_pool(name="ps_pj", bufs=4, space="PSUM") as ps_pj:
                for cs in range(4):
                    for c in range(16):
                        pj = ps_pj.tile([128, 512], FP32, tag="pj")
                        nc.tensor.matmul(
                            pj[:],
                            projT_sb[:, cs * 128 : (cs + 1) * 128],
                            y_sb[:, c * 512 : (c + 1) * 512],
                            start=True, stop=True,
                        )
                        o_t = opool.tile([128, 512], FP16, tag="o_t")
                        if (cs * 16 + c) % 2 == 0:
                            nc.vector.tensor_copy(o_t[:], pj[:])
                        else:
                            nc.scalar.copy(o_t[:], pj[:])
                        nc.scalar.dma_start(
                            out=outT[cs * 128 : (cs + 1) * 128,
                                     c * 512 : (c + 1) * 512],
                            in_=o_t[:],
                        )
    nc.compile()
    return nc


def _prep_inputs(x, qkv_w, qkv_b, rpb, proj_w, proj_b):
    x = np.asarray(x, np.float32)
    qkv_w = np.asarray(qkv_w, np.float32)
    qkv_b = np.asarray(qkv_b, np.float32)
    rpb = np.asarray(rpb, np.float32)
    proj_w = np.asarray(proj_w, np.float32)

    xT16 = np.ascontiguousarray(x.reshape(T, C).T).astype(np.float16)
    mi = (np.arange(N) // S)[:, None]
    mj = (np.arange(N) % S)[:, None]
    ni = (np.arange(N) // S)[None, :]
    nj = (np.arange(N) % S)[None, :]

    in_maps = []
    for core in range(N_CORES):
        h0, h1 = 2 * core, 2 * core + 1
        rq = list(range(h0 * D, h0 * D + D)) + list(range(h1 * D, h1 * D + D))
        wq = qkv_w[rq, :] * SCALE
        wk = qkv_w[[C + r for r in rq], :]
        wv = qkv_w[[2 * C + r for r in rq], :]
        bq = qkv_b[rq] * SCALE
        bk = qkv_b[[C + r for r in rq]]
        bv = qkv_b[[2 * C + r for r in rq]]

        wqk = np.concatenate([wq, wk], axis=0)           # (128, 512)
        wqkT16 = np.ascontiguousarray(wqk.T).astype(np.float16).reshape(4, 128, 128)
        wvT16 = np.ascontiguousarray(wv.T).astype(np.float16).reshape(4, 128, 64)
        bqk_in = np.concatenate([bq, bk]).astype(np.float32).reshape(128, 1)
        bv2_in = np.concatenate([bv, bv]).astype(np.float32).reshape(128, 1)

        expb_in = np.zeros((128, 16384), np.float16)
        for hi, h in enumerate((h0, h1)):
            biasT = rpb[h][31 + mi - ni, 31 + mj - nj]    # (m, n) = bias^T
            for half in range(2):
                blk = biasT[:, half * 512 : (half + 1) * 512]
                blk = blk.reshape(8, 128, 512).transpose(1, 0, 2)  # (128, j, 512)
                blk = blk[:, [0, 2, 4, 6, 1, 3, 5, 7], :].reshape(128, 4096)
                expb_in[:, (hi * 2 + half) * 4096 : (hi * 2 + half + 1) * 4096] = (
                    np.exp(blk).astype(np.float16)
                )

        projT_in = np.zeros((128, 512), np.float16)
        projT_in[0:32] = proj_w[:, 64 * core : 64 * core + 32].T.astype(np.float16)
        projT_in[64:96] = proj_w[:, 64 * core + 32 : 64 * core + 64].T.astype(np.float16)

        in_maps.append({
            "xT": xT16,
            "wqkT": wqkT16,
            "wvT": wvT16,
            "bqk": bqk_in,
            "bv2": bv2_in,
            "expb": expb_in,
            "projT": projT_in,
        })
    return in_maps


def kernel(x, qkv_w, qkv_b, rpb, proj_w, proj_b):
    if "nc" not in _CACHE:
        _CACHE["nc"] = build_nc()
    nc = _CACHE["nc"]
    in_maps = _prep_inputs(x, qkv_w, qkv_b, rpb, proj_w, proj_b)
    res = run_bass_kernel_spmd(nc, in_maps, list(range(N_CORES)))
    out = np.zeros((T, C), np.float32)
    for core in range(N_CORES):
        out += res.results[core]["outT"].astype(np.float32).T
    out += np.asarray(proj_b, np.float32)[None, :]
    return out.reshape(B, S, S, C)


if __name__ == "__main__":
    rng = np.random.default_rng(0)
    ins = {
        "x": rng.standard_normal((B, S, S, C)).astype(np.float32),
        "qkv_w": (rng.standard_normal((3 * C, C)) * 0.02).astype(np.float32),
        "qkv_b": (rng.standard_normal((3 * C,)) * 0.02).astype(np.float32),
        "rpb": (rng.standard_normal((NH, 2 * S - 1, 2 * S - 1)) * 0.02).astype(np.float32),
        "proj_w": (rng.standard_normal((C, C)) * 0.02).astype(np.float32),
        "proj_b": (rng.standard_normal((C,)) * 0.02).astype(np.float32),
    }
    out = kernel(**ins)
    print("kernel ran, out", out.shape, out.dtype, float(np.abs(out).max()))
